# revision 11
# baseline (speedup 1.0000x reference)
"""Differentiable top-k (Sinkhorn) Trainium2 kernel.

Math: the reference runs 100 log-domain Sinkhorn iterations on
log_P0[i,j] = -(s_i - sorted_j)^2/eps, then sums exp(log_P) over the
first K=50 columns.

Equivalent multiplicative form used here: relabel rows by descending
rank so the kernel matrix Kt[a,b] = exp(-(t_a - t_b)^2/eps) (t = sorted
scores) is symmetric.  The alternating column/row normalizations become
a single chain  w_{k+1} = 1 / (Kt @ w_k),  w_0 = 1  (u_T = w_{2T-1},
v_T = w_{2T}).  Final:  out_sorted[a] = v[a] * sum_{b<50} Kt[a,b] u[b],
out[i] = out_sorted[rank_i].

Per core: 2 independent batches.  Kt is stored as fp16 PE weight tiles
(fast-weight-load); each matvec is <=16 accumulating [128,128]x[128,1]
matmuls (only band-blocks that contain any |t_a-t_b| <= 0.296 are
emitted -- entries beyond that underflow fp32's exp to exact 0).
Reciprocals run on the vector engine.  The sort itself is done on-chip
with comparison-count ranks and permutation-matrix matmuls.
"""

import numpy as np

import concourse.bacc as bacc
import concourse.mybir as mybir
from concourse import tile
from concourse.bass_utils import run_bass_kernel_spmd

F32 = mybir.dt.float32
F16 = mybir.dt.float16

B_FULL = 16
N = 512
NB = N // 128  # 4 column blocks
TK = 50
EPS = 1e-3
T_ITERS = 100  # Sinkhorn iterations (2*T matvec steps)
N_CORES = 8
B_LOC = B_FULL // N_CORES  # batches per core
# |t_a - t_b| beyond this gives exp(-d^2/eps) < 1e-38 == fp32 0
D_CUT = float(np.sqrt(87.5 * EPS))
SQRT_SCALE = float(np.sqrt(1.0 / EPS))


def _band_blocks(scores):
    """128-block band structure of the sorted-score kernel matrix,
    unioned over all batches (one SPMD program runs on every core)."""
    t = -np.sort(-scores.astype(np.float64), axis=-1)
    need = set()
    for b in range(scores.shape[0]):
        tb = t[b]
        hi = [tb[c * 128] for c in range(NB)]        # block max (descending)
        lo = [tb[c * 128 + 127] for c in range(NB)]  # block min
        for io in range(NB):
            for jo in range(NB):
                gap = max(0.0, max(lo[io] - hi[jo], lo[jo] - hi[io]))
                if gap <= D_CUT:
                    need.add((io, jo))
    blocks = {io: sorted(jo for (i, jo) in need if i == io) for io in range(NB)}
    for io in range(NB):
        assert io in blocks[io]
    return blocks


def _build(blocks, t_iters):
    nc = bacc.Bacc("TRN2", target_bir_lowering=False, debug=False)

    scores_d = nc.declare_dram_parameter("scores", [B_LOC, N], F32, isOutput=False)
    iota_rep_d = nc.declare_dram_parameter("iota_rep", [128, N], F32, isOutput=False)
    iota_part_d = nc.declare_dram_parameter("iota_part", [128, NB], F32, isOutput=False)
    ones_row_d = nc.declare_dram_parameter("ones_row", [1, 128], F32, isOutput=False)
    neg_ones_row_d = nc.declare_dram_parameter("neg_ones_row", [1, 128], F32, isOutput=False)
    mask50_d = nc.declare_dram_parameter("mask50", [128, 1], F16, isOutput=False)
    out_d = nc.declare_dram_parameter("out", [B_LOC, N], F32, isOutput=True)

    with nc.allow_low_precision(reason="fp16 sinkhorn iterates"), \
         tile.TileContext(nc) as tc:
        with tc.tile_pool(name="sb", bufs=1) as sb, \
             tc.tile_pool(name="scr", bufs=4) as scr, \
             tc.tile_pool(name="wp", bufs=2) as wp, \
             tc.tile_pool(name="ps_big", bufs=2, space="PSUM") as ps_big, \
             tc.tile_pool(name="ps_row", bufs=2, space="PSUM") as ps_row, \
             tc.tile_pool(name="ps_w", bufs=2, space="PSUM") as ps_w:

            # constants
            iota_rep = sb.tile([128, N], F32, tag="iota_rep")
            iota_part = sb.tile([128, NB], F32, tag="iota_part")
            ones_row = sb.tile([1, 128], F32, tag="ones_row")
            neg_ones_row = sb.tile([1, 128], F32, tag="neg_ones_row")
            mask50 = sb.tile([128, 1], F16, tag="mask50")
            zero_col = sb.tile([128, 1], F32, tag="zero_col")
            nc.vector.memset(zero_col[:], 0.0)
            nc.sync.dma_start(iota_rep[:], iota_rep_d[:])
            nc.sync.dma_start(iota_part[:], iota_part_d[:])
            nc.sync.dma_start(ones_row[:], ones_row_d[:])
            nc.sync.dma_start(neg_ones_row[:], neg_ones_row_d[:])
            nc.sync.dma_start(mask50[:], mask50_d[:])

            kw = {}    # kw[(b, jo)] : fp16 [128, N] weight tiles of Kt
            pmt = {}   # pmt[(b, ro)]: fp32 [128, N] transposed permutation
            uvw = {}   # final u32/v32 per batch
            rank_rows = {}

            for b in range(B_LOC):
                # ---- load scores in row + partition-major layouts ----
                s_row = sb.tile([1, N], F32, tag=f"s_row{b}")
                s_part = sb.tile([128, NB], F32, tag=f"s_part{b}")
                nc.sync.dma_start(
                    s_row[:], scores_d[b].rearrange("(o n) -> o n", o=1)
                )
                nc.sync.dma_start(
                    s_part[:], scores_d[b].rearrange("(c p) -> p c", p=128)
                )

                # ---- ranks: rank_i = #{j : s_j > s_i} ----
                # cmp[c][p, i] = (s_i > s_{c*128+p});
                #   free-sum  -> #{i beaten by this j}  (unused)
                #   accum_out -> rank_part via per-partition counts? no:
                # accum gives sum_i over FREE i of (s_i > s_j) = #{i: s_i > s_j}
                # = rank of j. partition j = c*128+p -> rank_part[p, c]. OK.
                s_rep = ps_big.tile([128, N], F32, tag="ps_big")
                nc.tensor.matmul(s_rep[:], ones_row[:], s_row[:])  # replicate row
                rank_part = sb.tile([128, NB], F32, tag=f"rank_part{b}")
                cmps = []
                for c in range(NB):
                    cm = scr.tile([128, N], F32, tag="cmp")
                    nc.vector.tensor_scalar(
                        out=cm[:],
                        in0=s_rep[:],
                        scalar1=s_part[:, c : c + 1],
                        scalar2=0.0,
                        op0=mybir.AluOpType.is_gt,
                        op1=mybir.AluOpType.add,
                        accum_out=rank_part[:, c : c + 1],
                    )
                    cmps.append(cm)
                rank_row = sb.tile([1, N], F32, tag=f"rank_row{b}")
                rank_rows[b] = rank_row
                for c in range(NB):
                    nc.sync.dma_start(
                        rank_row[:, c * 128 : (c + 1) * 128],
                        rank_part[:, c : c + 1],
                    )

                # ---- sorted scores: t_row, t_part via permutation matmuls ----
                pms = []
                t_row_ps = ps_row.tile([1, N], F32, tag="ps_row")
                for c in range(NB):
                    pm = scr.tile([128, N], F32, tag="pm")
                    nc.vector.tensor_scalar(
                        out=pm[:],
                        in0=iota_rep[:],
                        scalar1=rank_part[:, c : c + 1],
                        scalar2=None,
                        op0=mybir.AluOpType.is_equal,
                    )
                    pms.append(pm)
                    nc.tensor.matmul(
                        t_row_ps[:],
                        s_part[:, c : c + 1],
                        pm[:],
                        start=(c == 0),
                        stop=(c == NB - 1),
                    )
                t_row = sb.tile([1, N], F32, tag=f"t_row{b}")
                nc.scalar.copy(t_row[:], t_row_ps[:])
                t_part = sb.tile([128, NB], F32, tag=f"t_part{b}")
                for c in range(NB):
                    nc.sync.dma_start(
                        t_part[:, c : c + 1],
                        t_row[:, c * 128 : (c + 1) * 128],
                    )

                # ---- Kt weight tiles (fp16) ----
                # arg = 2000*t_a*t_b - 1000*t_b^2 - 1000*t_a^2 = -1000(t_a-t_b)^2
                # built as two K=1 outer-product matmuls + Exp with bias.
                t2000_row = sb.tile([1, N], F32, tag=f"t2000_{b}")
                nc.scalar.activation(
                    t2000_row[:], t_row[:],
                    mybir.ActivationFunctionType.Copy, scale=2000.0,
                )
                post2_row = sb.tile([1, N], F32, tag=f"post2_{b}")
                nc.scalar.activation(
                    post2_row[:], t_row[:],
                    mybir.ActivationFunctionType.Square,
                    bias=zero_col[0:1, 0:1], scale=float(np.sqrt(1000.0)),
                )
                negt2_part = sb.tile([128, NB], F32, tag=f"negt2_{b}")
                for c in range(NB):
                    nc.vector.tensor_scalar(
                        out=negt2_part[:, c : c + 1],
                        in0=t_part[:, c : c + 1],
                        scalar1=t_part[:, c : c + 1],
                        scalar2=-1000.0,
                        op0=mybir.AluOpType.mult,
                        op1=mybir.AluOpType.mult,
                    )
                for jo in range(NB):
                    kb_ps = ps_big.tile([128, N], F32, tag="ps_big")
                    nc.tensor.matmul(
                        kb_ps[:], t_row[:, jo * 128 : (jo + 1) * 128],
                        t2000_row[:], start=True, stop=False,
                    )
                    nc.tensor.matmul(
                        kb_ps[:], neg_ones_row[:], post2_row[:],
                        start=False, stop=True,
                    )
                    kt = sb.tile([128, N], F16, tag=f"kt{b}_{jo}")
                    nc.scalar.activation(
                        kt[:], kb_ps[:], mybir.ActivationFunctionType.Exp,
                        bias=negt2_part[:, jo : jo + 1], scale=1.0,
                    )
                    kw[(b, jo)] = kt

            # ---- 2*T matvec steps, batches interleaved ----
            w16 = {}
            for b in range(B_LOC):
                w0 = wp.tile([128, NB], F16, tag=f"w{b}")
                nc.vector.memset(w0[:], 1.0)
                w16[b] = w0

            n_steps = 2 * t_iters
            for k in range(n_steps):
                for b in range(B_LOC):
                    pw = ps_w.tile([128, NB], F32, tag=f"pw{b}")
                    for io in range(NB):
                        jos = blocks[io]
                        for ji, jo in enumerate(jos):
                            nc.tensor.matmul(
                                pw[:, io : io + 1],
                                kw[(b, jo)][:, io * 128 : (io + 1) * 128],
                                w16[b][:, jo : jo + 1],
                                start=(ji == 0),
                                stop=(ji == len(jos) - 1),
                            )
                    if k == n_steps - 2:
                        u32 = sb.tile([128, NB], F32, tag=f"u32_{b}")
                        nc.vector.reciprocal(u32[:], pw[:])
                        wn = wp.tile([128, NB], F16, tag=f"w{b}")
                        nc.vector.reciprocal(wn[:], pw[:])
                        uvw[(b, "u32")] = u32
                    elif k == n_steps - 1:
                        v32 = sb.tile([128, NB], F32, tag=f"v32_{b}")
                        nc.vector.reciprocal(v32[:], pw[:])
                        wn = w16[b]  # unused afterwards
                        uvw[(b, "v32")] = v32
                    else:
                        wn = wp.tile([128, NB], F16, tag=f"w{b}")
                        nc.vector.reciprocal(wn[:], pw[:])
                    w16[b] = wn

            # ---- transposed permutation PmT for the final unpermute ----
            for b in range(B_LOC):
                rank_rep_ps = ps_big.tile([128, N], F32, tag="ps_big")
                nc.tensor.matmul(
                    rank_rep_ps[:], ones_row[:], rank_rows[b][:]
                )
                rank_rep = scr.tile([128, N], F32, tag="rank_rep")
                nc.scalar.copy(rank_rep[:], rank_rep_ps[:])
                for ro in range(NB):
                    pt = sb.tile([128, N], F32, tag=f"pmt{b}_{ro}")
                    nc.vector.tensor_scalar(
                        out=pt[:],
                        in0=rank_rep[:],
                        scalar1=iota_part[:, ro : ro + 1],
                        scalar2=None,
                        op0=mybir.AluOpType.is_equal,
                    )
                    pmt[(b, ro)] = pt

            # ---- output: os = v * (Kt[:, :50] @ u), unpermute, store ----
            for b in range(B_LOC):
                u50 = sb.tile([128, 1], F16, tag=f"u50_{b}")
                nc.vector.tensor_tensor(
                    out=u50[:], in0=w16[b][:, 0:1], in1=mask50[:],
                    op=mybir.AluOpType.mult,
                )
                o50 = ps_w.tile([128, NB], F32, tag=f"pw{b}")
                for io in range(NB):
                    nc.tensor.matmul(
                        o50[:, io : io + 1],
                        kw[(b, 0)][:, io * 128 : (io + 1) * 128],
                        u50[:],
                        start=True,
                        stop=True,
                    )
                os_part = sb.tile([128, NB], F32, tag=f"os_{b}")
                nc.vector.tensor_tensor(
                    out=os_part[:], in0=o50[:], in1=uvw[(b, "v32")][:],
                    op=mybir.AluOpType.mult,
                )
                orow = ps_row.tile([1, N], F32, tag="ps_row")
                for ro in range(NB):
                    nc.tensor.matmul(
                        orow[:],
                        os_part[:, ro : ro + 1],
                        pmt[(b, ro)][:],
                        start=(ro == 0),
                        stop=(ro == NB - 1),
                    )
                orow_sb = sb.tile([1, N], F32, tag=f"orow_{b}")
                nc.scalar.copy(orow_sb[:], orow[:])
                nc.sync.dma_start(
                    out_d[b].rearrange("(o n) -> o n", o=1), orow_sb[:]
                )

    nc.compile()
    return nc


def kernel(scores):
    scores = np.ascontiguousarray(np.asarray(scores, dtype=np.float32))
    assert scores.shape == (B_FULL, N)
    blocks = _band_blocks(scores)
    nc = _build(blocks, T_ITERS)

    iota_rep = np.broadcast_to(np.arange(N, dtype=np.float32), (128, N)).copy()
    iota_part = np.arange(N, dtype=np.float32).reshape(NB, 128).T.copy()
    ones_row = np.ones((1, 128), np.float32)
    neg_ones_row = -np.ones((1, 128), np.float32)
    mask50 = np.zeros((128, 1), np.float16)
    mask50[:TK] = 1.0

    in_maps = []
    for c in range(N_CORES):
        in_maps.append(
            {
                "scores": scores[c * B_LOC : (c + 1) * B_LOC],
                "iota_rep": iota_rep,
                "iota_part": iota_part,
                "ones_row": ones_row,
                "neg_ones_row": neg_ones_row,
                "mask50": mask50,
            }
        )
    res = run_bass_kernel_spmd(nc, in_maps, core_ids=list(range(N_CORES)))
    return np.concatenate(
        [res.results[c]["out"] for c in range(N_CORES)], axis=0
    ).astype(np.float32)


if __name__ == "__main__":
    scores = np.load("/root/problem/scores.npy")
    out = kernel(scores)
    exp = np.load("/root/problem/expected.npy")
    err = ((out - exp) ** 2).mean() / ((exp**2).mean() + 1e-8)
    print("resid_var:", err, "absmax:", np.abs(out - exp).max())
